# revision 4
# baseline (speedup 1.0000x reference)
"""Self-contained Trainium2 Bass kernel for single-head T2T attention (v2, fp8).

Problem: x:[8,4096,512], w_qkv:[1536,512], w_proj:[512,512], b_proj:[512]
    qkv = x @ w_qkv.T ; q,k,v split
    attn = softmax(q @ k.T / sqrt(512))
    out  = v + (attn @ v) @ w_proj.T + b_proj

Sharding: data-parallel over batch B=8 across the 8 NeuronCores (one
example per core); weights replicated.  No collectives needed.

v2 strategy (vs the fp32r/bf16 v1): the output is v + o where |o|/|v| ~ 0.7%
for this input distribution, so the attention path tolerates fp8 easily while
v (the residual) is kept at fp32r accuracy.  All big matmuls except the V
projection run as float8e4 with MatmulPerfMode.DoubleRow: each instruction
contracts TWO 128-row k-tiles ([K,2,M] lhsT / [K,2,N] rhs) at 0.5 cycles
per output row -- 4x fewer PE cycles than bf16 for the same math.

Scale folding (no extra instructions, keeps fp8 operands in range):
    wqk8    = fp8(16 * w_qkv[0:1024])        -> Qh=16Q, Kh=16K  (std ~7)
    scores  Sh = Qh.Kh = 256*S               -> exp scale = SCALE/256
    exp     Ph = exp(Sh*scale + ln 64) = 64*P  (range ~[24, 180] in fp8e4)
    v8      = fp8(V)                          (std ~0.45)
    ot      = sum Ph*v8 = 64*(P@V)            -> oT8 = fp8(64*O), std ~0.46
    wproj8  = fp8(16 * w_proj)                -> pj = 1024*(O@Wp)
    ones16  = 16                              -> sums = 1024*sum(P)
    fin     = pj * (1/sums) + vres  ==  (P@V@Wp)/sum(P) + v   (exact folding)

Per-core dataflow (N=4096, C=512, P=128):
  phase 0: PE-transpose weights into wqk8 [c,2C] fp8, wvr [c,C] f32r,
      wproj8 [d,C] fp8 (x16 scale applied during the PSUM->SBUF copies).
  phase 1 (per 512-wide n-chunk): stream x, PE-transpose to x^T (fp32),
      copy to xTr f32r (ACT) and xT8 fp8 (Pool); V = x@wv in f32r with
      fp8 copy (ACT) + fp32(+bias) residual copy (DVE/Pool);
      Q^T,K^T via fp8 DoubleRow, fp8 copies into resident qT8/kT8 (DVE).
      Everything stays in SBUF -- no DRAM scratch.
  phase 2 (per 512-wide query chunk): m-loop over 16 m-block PAIRS:
      S^T pair-block via 4 DoubleRow matmuls into a [128,2,512] PSUM tile,
      ONE exp activation per pair ([128,1024], scores bounded so softmax
      without max-subtraction is safe), PV via 4 DoubleRow matmuls
      accumulating O^T in 4 PSUM banks.  The m-loop is software-pipelined
      (PV one pair behind exp).  Denominators: 64 tiny DoubleRow matmuls
      against ones16 AFTER the m-loop (pT_all stays resident), giving
      per-row sums as columns directly; DVE reciprocal; normalization is
      folded into the final scalar_tensor_tensor (it commutes with the
      row-wise linear proj; bias is pre-added into the vres copies).
"""

import numpy as np

import concourse.bass as bass
import concourse.mybir as mybir
from concourse.tile import TileContext
from concourse.masks import make_identity

P = 128
B = 8
N_FULL = 4096
C = 512
F = 3 * C
NQ = 512           # query chunk width (free dim of most matmuls)
CB = C // P        # 4 contraction sub-blocks of the model dim
SCALE = 1.0 / float(np.sqrt(C))
F32 = mybir.dt.float32
F32R = mybir.dt.float32r
FP8 = mybir.dt.float8e4
DR = mybir.MatmulPerfMode.DoubleRow

WS = 16.0          # weight pre-scale for w_qk / w_proj fp8 casts
ES = 64.0          # exp output scale, applied via bias = ln(ES)
OS = 1.0 / 64.0    # scale on the O^T psum->fp8 copy (keeps |sum P*V| < fp8 max)
ONEV = ES * OS * WS / ES   # denominator const so recip folds exactly: 0.25


# ---------------------------------------------------------------------------
# Workaround: this container's walrus build accepts at most one sync wait per
# plain instruction (two for EventSemaphore), but Tile's wait assignment can
# attach several.  Post-pass: move excess waits onto injected same-engine
# NOPs placed immediately before the over-subscribed instruction.
# ---------------------------------------------------------------------------
def _legalize_waits(nc):
    for fn in nc.m.functions:
        for bb in fn.blocks:
            insts = bb.instructions
            out = []
            changed = False
            for inst in insts:
                si = inst.sync_info
                waits = list(si.on_wait) if si and si.on_wait else []
                cap = 2 if isinstance(inst, mybir.InstEventSemaphore) else 1
                if len(waits) > cap:
                    keep = waits[:cap]
                    rest = waits[cap:]
                    for i, w in enumerate(rest):
                        nop = mybir.InstNoOp(
                            name=f"{inst.name}-wspill{i}",
                            ins=[], outs=[], engine=inst.engine)
                        nop.sync_info = mybir.SyncInfo(
                            on_wait=[w], on_update=[])
                        nc.register_instruction(nop, overwrite=True)
                        out.append(nop)
                    si.on_wait = keep
                    changed = True
                out.append(inst)
            if changed:
                insts.clear()
                insts.extend(out)


class _nullctx:
    def __enter__(self):
        return None

    def __exit__(self, *a):
        return False


def build_program(n=N_FULL, reps=1, hw_loop=0, has_bias=False, variant="all"):
    """Build the per-core Bass program for one [n, C] example."""
    n_chunks = n // NQ
    mb_total = n // P
    npair = mb_total // 2

    nc = bass.Bass("TRN2", target_bir_lowering=False,
                   dynamic_dma_scratch_size=8192)
    x = nc.dram_tensor("x", (n, C), F32, kind="ExternalInput")
    w_qkv = nc.dram_tensor("w_qkv", (F, C), F32, kind="ExternalInput")
    w_proj = nc.dram_tensor("w_proj", (C, C), F32, kind="ExternalInput")
    b_proj = nc.dram_tensor("b_proj", (C,), F32, kind="ExternalInput")
    out = nc.dram_tensor("out", (n, C), F32, kind="ExternalOutput")

    def f32view(ap):
        # fp32r storage is fp32 bits; view as fp32 for non-PE ops
        return ap.bitcast(F32) if ap.dtype == F32R else ap

    with TileContext(nc) as tc:
        with tc.tile_pool(name="singles", bufs=1) as singles:
            ident = singles.tile([P, P], F32)
            make_identity(nc, ident)
            ones16 = singles.tile([P, 2, 1], FP8)
            nc.vector.memset(ones16, ONEV)
            expbias = singles.tile([P, 1], F32)
            nc.vector.memset(expbias, float(np.log(ES)))
            bias_bc = singles.tile([P, C], F32)
            nc.sync.dma_start(
                out=bias_bc, in_=b_proj[:].unsqueeze(0).to_broadcast((P, C)))

            qT8 = singles.tile([P, CB, n], FP8)      # Q^T: [d, n] fp8 (x16)
            kT8 = singles.tile([P, CB, n], FP8)      # K^T: [d, m] fp8 (x16)
            v8 = singles.tile([P, mb_total, C], FP8)   # V: [m, d] fp8
            vres = singles.tile([P, mb_total, C], F32)  # V + bias, exact
            wqk8 = singles.tile([P, CB, 2 * C], FP8)   # [c, f] fp8 (x16)
            wvr = singles.tile([P, CB, C], F32R)       # [c, d] f32r
            wproj8 = singles.tile([P, CB, C], FP8)     # [d, e] fp8 (x16)

            rep_ctx = (tc.For_i(0, hw_loop, 1) if hw_loop
                       else _nullctx())
            with rep_ctx:
              for _rep in range(reps):
                # ---- phase 0 + 1: weight transposes, x^T, QKV ----
                with tc.tile_pool(name="wload", bufs=3) as wload, \
                     tc.tile_pool(name="xtr", bufs=2) as xtr_pool, \
                     tc.tile_pool(name="xt8", bufs=2) as xt8_pool, \
                     tc.tile_pool(name="tp_psum", bufs=2, space="PSUM") as tp_psum, \
                     tc.tile_pool(name="qk_psum", bufs=2, space="PSUM") as qk_psum, \
                     tc.tile_pool(name="v_psum", bufs=2, space="PSUM") as v_psum:

                    for rb in range(F // P):          # 12 w_qkv row blocks
                        wnat = wload.tile([P, C], F32, tag="wnat")
                        nc.sync.dma_start(out=wnat, in_=w_qkv[rb * P:(rb + 1) * P, :])
                        tpw = tp_psum.tile([P, C], F32, tag="tp")
                        for cb in range(CB):
                            nc.tensor.transpose(
                                tpw[:, cb * P:(cb + 1) * P],
                                wnat[:, cb * P:(cb + 1) * P], ident)
                        if rb < 8:                    # Q,K rows -> fp8 x16
                            eng = nc.scalar if rb % 2 == 0 else nc.vector
                            if eng is nc.scalar:
                                eng.mul(wqk8[:, :, rb * P:(rb + 1) * P], tpw, WS)
                            else:
                                eng.tensor_scalar_mul(
                                    out=wqk8[:, :, rb * P:(rb + 1) * P],
                                    in0=tpw, scalar1=WS)
                        else:                         # V rows -> f32r exact
                            nc.scalar.copy(
                                out=wvr[:, :, (rb - 8) * P:(rb - 7) * P], in_=tpw)
                    for eb in range(C // P):          # 4 w_proj row blocks
                        wnat = wload.tile([P, C], F32, tag="wnat")
                        nc.sync.dma_start(out=wnat, in_=w_proj[eb * P:(eb + 1) * P, :])
                        tpw = tp_psum.tile([P, C], F32, tag="tp")
                        for db in range(CB):
                            nc.tensor.transpose(
                                tpw[:, db * P:(db + 1) * P],
                                wnat[:, db * P:(db + 1) * P], ident)
                        nc.vector.tensor_scalar_mul(
                            out=wproj8[:, :, eb * P:(eb + 1) * P],
                            in0=tpw, scalar1=WS)

                    for ch in range(n_chunks):
                        n0 = ch * NQ
                        xTr = xtr_pool.tile([P, CB, NQ], F32R, tag="xtr")
                        xT8 = xt8_pool.tile([P, CB, NQ], FP8, tag="xt8")
                        for nb in range(NQ // P):
                            xn = wload.tile([P, C], F32, tag="xn")
                            nc.sync.dma_start(
                                out=xn, in_=x[n0 + nb * P:n0 + (nb + 1) * P, :])
                            tp = tp_psum.tile([P, C], F32, tag="tp")
                            for cb in range(CB):
                                nc.tensor.transpose(
                                    tp[:, cb * P:(cb + 1) * P],
                                    xn[:, cb * P:(cb + 1) * P], ident)
                            nc.scalar.copy(
                                out=xTr[:, :, nb * P:(nb + 1) * P], in_=tp)
                            nc.gpsimd.tensor_copy(
                                out=xT8[:, :, nb * P:(nb + 1) * P],
                                in_=f32view(xTr[:, :, nb * P:(nb + 1) * P]))
                        # V (f32r, accuracy-critical residual); nb-pairs are
                        # interleaved across the cb chain so consecutive
                        # matmuls never hit the same PSUM region
                        for nb0 in range(0, NQ // P, 2):
                            vps = [v_psum.tile([P, NQ], F32, tag="v",
                                               name=f"vp{i}")
                                   for i in range(2)]
                            for cb in range(CB):
                                for i in range(2):
                                    nc.tensor.matmul(
                                        vps[i],
                                        xTr[:, cb, (nb0 + i) * P:(nb0 + i + 1) * P],
                                        wvr[:, cb, :],
                                        start=(cb == 0), stop=(cb == CB - 1))
                            for i in range(2):
                              nb = nb0 + i
                              vp = vps[i]
                              nc.scalar.copy(out=v8[:, ch * (NQ // P) + nb, :], in_=vp)
                            # Pool cannot touch PSUM.  ACT cannot apply a
                            # per-column bias, so with a bias all residual
                            # adds go to DVE; the common b_proj==0 case
                            # splits plain copies between DVE and ACT.
                              vdst = vres[:, ch * (NQ // P) + nb, :]
                              if has_bias:
                                  nc.vector.tensor_add(out=vdst, in0=vp, in1=bias_bc)
                              elif nb % 2 == 0:
                                  nc.vector.tensor_copy(out=vdst, in_=vp)
                              else:
                                  nc.scalar.copy(out=vdst, in_=vp)
                        # Q^T,K^T (fp8 DoubleRow)
                        for fp_ in range(4):
                            qkp = qk_psum.tile([P, 2, NQ], F32, tag="qk")
                            for h in range(2):
                                fb = 2 * fp_ + h
                                for ci in range(2):
                                    nc.tensor.matmul(
                                        qkp[:, h, :],
                                        wqk8[:, 2 * ci:2 * ci + 2, fb * P:(fb + 1) * P],
                                        xT8[:, 2 * ci:2 * ci + 2, :],
                                        start=(ci == 0), stop=(ci == 1),
                                        perf_mode=DR)
                            tgt = qT8 if fp_ < 2 else kT8
                            blk = (2 * fp_) % 4
                            nc.vector.tensor_copy(
                                out=tgt[:, blk:blk + 2, n0:n0 + NQ], in_=qkp)

                # ---- phase 2: attention + proj + residual ----
                if variant == "p1":
                    continue
                with tc.tile_pool(name="pT", bufs=2) as pT_pool, \
                     tc.tile_pool(name="oT8", bufs=2) as oT8_pool, \
                     tc.tile_pool(name="fin", bufs=3) as fin_pool, \
                     tc.tile_pool(name="rs", bufs=2) as rs_pool, \
                     tc.tile_pool(name="st_psum", bufs=2, space="PSUM") as st_psum, \
                     tc.tile_pool(name="ot_psum", bufs=4, space="PSUM") as ot_psum:

                    for ch in range(n_chunks):
                        n0 = ch * NQ
                        pT_all = pT_pool.tile([P, mb_total, NQ], FP8, tag="pT")
                        ot = [ot_psum.tile([P, NQ], F32, tag="ot", name=f"ot{db}")
                              for db in range(CB)]

                        def emit_pv(j):
                            for db in range(CB):
                                nc.tensor.matmul(
                                    ot[db],
                                    v8[:, 2 * j:2 * j + 2, db * P:(db + 1) * P],
                                    pT_all[:, 2 * j:2 * j + 2, :],
                                    start=(j == 0), stop=(j == npair - 1),
                                    perf_mode=DR)

                        # software-pipelined m-pair loop: PV one pair behind
                        # exp so the PE never waits on the ACT exp
                        for j in range(npair):
                            st = st_psum.tile([P, 2, NQ], F32, tag="st")
                            for h in range(2):
                                mb = 2 * j + h
                                for ci in range(2):
                                    nc.tensor.matmul(
                                        st[:, h, :],
                                        kT8[:, 2 * ci:2 * ci + 2, mb * P:(mb + 1) * P],
                                        qT8[:, 2 * ci:2 * ci + 2, n0:n0 + NQ],
                                        start=(ci == 0), stop=(ci == 1),
                                        perf_mode=DR)
                            nc.scalar.activation(
                                out=pT_all[:, 2 * j:2 * j + 2, :], in_=st,
                                func=mybir.ActivationFunctionType.Exp,
                                scale=SCALE / (WS * WS),
                                bias=expbias)
                            if j >= 1:
                                emit_pv(j - 1)
                        emit_pv(npair - 1)

                        # denominators: tiny DoubleRow matmuls vs ones16 give
                        # per-row sums as column vectors directly
                        recip = rs_pool.tile([P, NQ // P], F32, tag="recip")
                        if variant == "nosums":
                            nc.vector.memset(recip, 2.4e-4)
                        else:
                            sums = st_psum.tile([P, NQ // P], F32, tag="st",
                                                name=f"sums{ch}")
                            for nb in range(NQ // P):
                                for j in range(npair):
                                    nc.tensor.matmul(
                                        sums[:, nb:nb + 1],
                                        pT_all[:, 2 * j:2 * j + 2, nb * P:(nb + 1) * P],
                                        ones16,
                                        start=(j == 0), stop=(j == npair - 1),
                                        perf_mode=DR)
                            nc.vector.reciprocal(out=recip, in_=sums)

                        oT8 = oT8_pool.tile([P, CB, NQ], FP8, tag="oT8")
                        for db in range(CB):
                            nc.vector.tensor_scalar_mul(
                                out=oT8[:, db, :], in0=ot[db], scalar1=OS)

                        for nb in range(NQ // P):
                            pj = st_psum.tile([P, C], F32, tag="st",
                                              name=f"pj{nb}")
                            for ci in range(2):
                                nc.tensor.matmul(
                                    pj,
                                    oT8[:, 2 * ci:2 * ci + 2, nb * P:(nb + 1) * P],
                                    wproj8[:, 2 * ci:2 * ci + 2, :],
                                    start=(ci == 0), stop=(ci == 1),
                                    perf_mode=DR)
                            fin = fin_pool.tile([P, C], F32, tag="fin")
                            # fin = pj * (1/rowsum) + (v + bias)
                            nc.vector.scalar_tensor_tensor(
                                out=fin, in0=pj,
                                scalar=recip[:, nb:nb + 1],
                                in1=vres[:, ch * (NQ // P) + nb, :],
                                op0=mybir.AluOpType.mult,
                                op1=mybir.AluOpType.add)
                            nc.sync.dma_start(
                                out=out[n0 + nb * P:n0 + (nb + 1) * P, :], in_=fin)
    _legalize_waits(nc)
    return nc


_PROGRAM_CACHE = {}


def _get_program(n=N_FULL, reps=1, has_bias=False):
    key = (n, reps, has_bias)
    if key not in _PROGRAM_CACHE:
        _PROGRAM_CACHE[key] = build_program(n, reps=reps, has_bias=has_bias)
    return _PROGRAM_CACHE[key]


def kernel(x, w_qkv, w_proj, b_proj):
    from concourse.bass_utils import run_bass_kernel_spmd

    x = np.ascontiguousarray(np.asarray(x, dtype=np.float32))
    w_qkv = np.ascontiguousarray(np.asarray(w_qkv, dtype=np.float32))
    w_proj = np.ascontiguousarray(np.asarray(w_proj, dtype=np.float32))
    b_proj = np.ascontiguousarray(np.asarray(b_proj, dtype=np.float32))
    b, n, c = x.shape
    assert (b, n, c) == (B, N_FULL, C)

    nc = _get_program(has_bias=bool(np.any(b_proj != 0.0)))
    in_maps = [
        {"x": x[i], "w_qkv": w_qkv, "w_proj": w_proj, "b_proj": b_proj}
        for i in range(B)
    ]
    res = run_bass_kernel_spmd(nc, in_maps, list(range(B)))
    return np.stack([res.results[i]["out"] for i in range(B)], axis=0)
